# revision 14
# baseline (speedup 1.0000x reference)
"""Trainium2 Bass kernel for nn_NegF1: distributed -F1 loss over 16.7M elements.

Data-parallel over 8 NeuronCores; each core streams its 2,097,152-element
slice of probs (f32) / lbls (int32) from HBM on the sync HWDGE ring.
Memory-bound: the stream runs at the HBM limit (~405 GB/s busy) and the
compute engines are hardware-throttled to ~half clock while it does, so
the design respects two hard constraints learned from profiles:

  * DMA dispatches are leashed ~8 deep by completion-semaphore lane
    recycling, and lanes recycle at the pace of the compute ops that
    consume each tile (~2.4us per DMA).  Every DMA therefore costs a
    dispatch slot at the tail of the stream -- keep the DMA count at the
    baseline's proven 2-per-tile budget and taper the last tiles so the
    late dispatches carry small transfers.
  * DVE runs at ~93% occupancy during the stream (2 passes over the
    data at the throttled clock); extra mid-stream DVE work starves the
    dispatch leash and stalls the stream itself.

Per [128, F] tile (identical to the proven baseline pipeline):
  ACT:  lb = fp8(l), fused accum -> Npos partial
  DVE:  stt z: (p > .5) + p -> comb[:, :F]      (plane only)
        stt y: (p > .5) * p -> comb[:, F:], fused accum -> Y partial
  PE:   fp8 DoubleRow diag trick: lhsT = lb 2-chunk, rhs = [z|y] 2-chunk,
        ALL 64 chunk-pairs accumulate into ONE PSUM bank (26us of chained
        matmuls, far under the 41us stream).

What changed vs the baseline (the endgame, worth ~3us):
  * The diag is folded ON-CHIP instead of evicted+DMAed: two DVE ops
    multiply the two 128-blocks of the PSUM accumulator by a [128,128]
    identity (shipped as a 64KB constant input on the scalar ring, with
    an early 1-column touch so its semaphore lane recycles immediately)
    with fused accumulate.  acc col gets ps[p, p] per partition.
  * ONE output DMA: every partial (Npos/Y per tile + the two folds)
    lands in disjoint columns of a single [128, 24] f32 accumulator
    tile.  No PSUM->SBUF eviction, no 128KB diag DMA, no 3-DMA drain.

Algebra: with g = [p > .5], z = g + p, y = g * p:
  TP  = trace(sum_c lb_c^T y_c)   SxC = trace(sum_c lb_c^T z_c) = Sx + C
  FP  = Y - TP;  FN = Npos - SxC + TP
Host combines partials in float64 and returns -f1 (rtol 2e-2 allows the
fp8 planes; measured rel err ~2e-6).
"""

from contextlib import ExitStack

import numpy as np

N_TOTAL = 16777216
N_CORES = 8
M_PER_CORE = N_TOTAL // N_CORES   # 2097152
P = 128                           # SBUF partitions
EPS = 1e-05
CH = 128                          # diag chunk columns

HEAD_P = 32                       # head tile partitions (scalar ring)
HEAD_F = 1024                     # head free dim: 32*1024 elems = 256 cols
TILES = [2048] * 7 + [1024, 512, 256]        # 16128 cols after the head
N_TILES = 1 + len(TILES)                      # head + 10 PE tiles = 11

# acc column layout (single [P, ACC_K] f32 output tile)
#   accN  : cols [0, N_TILES)
#   accY  : cols [N_TILES, 2*N_TILES)
#   folds : 2*N_TILES + {0: SxC, 1: TP}
ACC_K = 2 * N_TILES + 2
ACC_PAD = ACC_K

_CACHE = {}


def build_nc(M=M_PER_CORE, F=2048, comb_bufs=4, warmup_mms=12, debug=False):
    import concourse.bacc as bacc
    import concourse.mybir as mybir
    import concourse.tile as tile

    cols = M // P                 # 16384
    assert HEAD_P * HEAD_F // P + sum(TILES) == cols
    assert all(Ft % (2 * CH) == 0 for Ft in TILES)

    f32 = mybir.dt.float32
    i32 = mybir.dt.int32
    bf16 = mybir.dt.bfloat16
    f8 = mybir.dt.float8e4
    Alu = mybir.AluOpType
    Act = mybir.ActivationFunctionType

    nc = bacc.Bacc("TRN2", target_bir_lowering=False, debug=debug,
                   num_devices=N_CORES)

    probs = nc.dram_tensor("probs", [M], f32, kind="ExternalInput")
    lbls = nc.dram_tensor("lbls", [M], i32, kind="ExternalInput")
    ident_in = nc.dram_tensor("ident", [P, CH], f32, kind="ExternalInput")
    out_acc = nc.dram_tensor("out_acc", [P, ACC_PAD], f32,
                             kind="ExternalOutput")

    def tile_view(ap_flat, start_el, p, f):
        return ap_flat[start_el:start_el + p * f].rearrange(
            "(p f) -> p f", p=p, f=f)

    p1 = probs.ap()
    l1 = lbls.ap()

    with tile.TileContext(nc) as tc, ExitStack() as ctx:
        pin = ctx.enter_context(tc.tile_pool(name="pin", bufs=1))
        lin = ctx.enter_context(tc.tile_pool(name="lin", bufs=1))
        lbpool = ctx.enter_context(tc.tile_pool(name="lbpool", bufs=3))
        cpool = ctx.enter_context(tc.tile_pool(name="cpool", bufs=comb_bufs))
        accp = ctx.enter_context(tc.tile_pool(name="accp", bufs=1))
        psump = ctx.enter_context(tc.tile_pool(name="psump", bufs=1,
                                               space="PSUM"))

        acc = accp.tile([P, ACC_PAD], f32)   # every partial sum lives here
        cN, cY, cX = 0, N_TILES, 2 * N_TILES

        # Scalar-ring transfers (no sync-ring dispatch slots, and the
        # head's 32-descriptor DMAs finish HWDGE generation ~0.5us before
        # the sync ring's first 128-descriptor tile, so the stream's
        # first bytes land earlier): the [32, 1024] head pair, then the
        # fold identity constant.
        hp = pin.tile([HEAD_P, HEAD_F], f32, tag="hp")
        nc.scalar.dma_start(out=hp[:, :], in_=tile_view(p1, 0, HEAD_P, HEAD_F))
        hl = lin.tile([HEAD_P, HEAD_F], i32, tag="hl")
        nc.scalar.dma_start(out=hl[:, :], in_=tile_view(l1, 0, HEAD_P, HEAD_F))
        ident = accp.tile([P, CH], f32)
        nc.scalar.dma_start(out=ident[:], in_=ident_in.ap())

        # Phase 1: issue EVERY input DMA up-front, all on the SYNC ring,
        # interleaved probs/lbls per tile so the single FIFO delivers tile
        # pairs in order.  Each tile has its own statically-assigned slot,
        # so nothing ever waits on a slot release; the completion-sem
        # leash paces dispatches at the compute-consumer rate, which the
        # 2-DMAs-per-tile budget and the end taper are sized to.
        pts, lts = [], []
        off_el = HEAD_P * HEAD_F
        for t, Ft in enumerate(TILES):
            start_el = off_el
            off_el += P * Ft
            pt = pin.tile([P, Ft], f32, tag=f"pt{t}")
            nc.sync.dma_start(out=pt[:, :Ft],
                              in_=tile_view(p1, start_el, P, Ft))
            lt = lin.tile([P, Ft], i32, tag=f"lt{t}")
            nc.sync.dma_start(out=lt[:, :Ft],
                              in_=tile_view(l1, start_el, P, Ft))
            pts.append(pt)
            lts.append(lt)

        # single accumulating diag bank for all 64 chunk pairs
        ps_diag = psump.tile([P, 2 * CH], f32)

        # Warm the PE HAM clock-gate while the first input DMAs stream.
        if warmup_mms:
            wu = accp.tile([P, 2 * CH], bf16)
            nc.vector.memset(wu[:], 0.0)
            ps_wu = psump.tile([P, 2 * CH], f32)
            for i in range(warmup_mms):
                nc.tensor.matmul(ps_wu[:, :], wu[:, :CH], wu[:],
                                 start=(i == 0), stop=(i == warmup_mms - 1))

        # Early 1-column touch of ident on DVE: consumes its DMA
        # completion semaphore right after it lands so the sem lane
        # recycles for the input-stream dispatch leash.
        junk1 = accp.tile([P, 1], f32)
        nc.vector.scalar_tensor_tensor(
            out=junk1[:, :], in0=ident[:, 0:1], scalar=0.0,
            in1=ident[:, 0:1], op0=Alu.bypass, op1=Alu.mult)



        # Head compute: same cast/z/y pipeline on 32 partitions, and the
        # diag matmuls accumulate into the SAME ps_diag bank -- a
        # DoubleRow matmul with a [32, 2, 128] lhsT contracts 64 rows and
        # still emits a full [128, 256] diag block, so the fold and the
        # host combine see head and bulk identically.
        lb_h = accp.tile([HEAD_P, HEAD_F], f8)
        nc.scalar.activation(lb_h[:, :], hl[:, :], Act.Copy,
                             accum_out=acc[0:HEAD_P, cN:cN + 1])
        comb_h = accp.tile([HEAD_P, 2 * HEAD_F], f8)
        nc.vector.scalar_tensor_tensor(
            out=comb_h[:, :HEAD_F], in0=hp[:, :], scalar=0.5,
            in1=hp[:, :], op0=Alu.is_gt, op1=Alu.add)
        nc.vector.scalar_tensor_tensor(
            out=comb_h[:, HEAD_F:], in0=hp[:, :], scalar=0.5,
            in1=hp[:, :], op0=Alu.is_gt, op1=Alu.mult,
            accum_out=acc[0:HEAD_P, cY:cY + 1])
        NCh = HEAD_F // CH
        comb_h4 = comb_h[:].rearrange("p (s c x) -> p c s x", s=2,
                                      c=NCh, x=CH)
        lb_h3 = lb_h[:].rearrange("p (c x) -> p c x", c=NCh, x=CH)
        for c in range(0, NCh, 2):
            nc.tensor.matmul(
                ps_diag[:, :], lb_h3[:, c:c + 2],
                comb_h4[:, c:c + 2],
                start=(c == 0), stop=False,
                perf_mode=mybir.MatmulPerfMode.DoubleRow)

        nctot = sum(TILES) // CH // 2      # 63 bulk chunk pairs
        ci = 0

        # Phase 2: compute, chasing the stream.
        for t, Ft in enumerate(TILES):
            NCt = Ft // CH
            pt, lt = pts[t], lts[t]

            # ACT: lb = fp8(l) with fused accum -> Npos
            lb = lbpool.tile([P, F], f8, tag="lb")
            nc.scalar.activation(lb[:, :Ft], lt[:, :Ft], Act.Copy,
                                 accum_out=acc[:, cN + 1 + t:cN + 2 + t])

            # DVE: z = g + p plane; y = g * p plane with fused accum -> Y.
            # Both passes stay on DVE: offloading y to ACT (as Relu(z-1))
            # was measured SLOWER -- the cross-engine z->y hop serializes
            # each tile's chain and ACT becomes the semaphore-lane pacer.
            comb = cpool.tile([P, 2 * F], f8, tag="comb")
            nc.vector.scalar_tensor_tensor(
                out=comb[:, :Ft], in0=pt[:, :Ft], scalar=0.5,
                in1=pt[:, :Ft], op0=Alu.is_gt, op1=Alu.add)
            nc.vector.scalar_tensor_tensor(
                out=comb[:, F:F + Ft], in0=pt[:, :Ft], scalar=0.5,
                in1=pt[:, :Ft], op0=Alu.is_gt, op1=Alu.mult,
                accum_out=acc[:, cY + 1 + t:cY + 2 + t])

            # PE diag in fp8 DoubleRow mode: each matmul contracts TWO
            # chunks, accumulating both chunks' diag blocks at once.
            comb4 = comb[:].rearrange("p (s c x) -> p c s x", s=2,
                                      c=F // CH, x=CH)
            lb3 = lb[:].rearrange("p (c x) -> p c x", c=F // CH, x=CH)
            for c in range(0, NCt, 2):
                nc.tensor.matmul(
                    ps_diag[:, :], lb3[:, c:c + 2],
                    comb4[:, c:c + 2],
                    start=False,
                    stop=(ci == nctot - 1),
                    perf_mode=mybir.MatmulPerfMode.DoubleRow)
                ci += 1
        assert ci == nctot

        # Diag folds on DVE, straight out of PSUM: acc col gets the
        # per-partition diagonal ps[p, p] of each 128-block; host sums.
        junk_g = accp.tile([P, CH], f32)
        nc.vector.scalar_tensor_tensor(
            out=junk_g[:, :], in0=ps_diag[:, :CH], scalar=0.0,
            in1=ident[:], op0=Alu.bypass, op1=Alu.mult,
            accum_out=acc[:, cX:cX + 1])
        nc.vector.scalar_tensor_tensor(
            out=junk_g[:, :], in0=ps_diag[:, CH:], scalar=0.0,
            in1=ident[:], op0=Alu.bypass, op1=Alu.mult,
            accum_out=acc[:, cX + 1:cX + 2])

        # single output DMA
        nc.sync.dma_start(out=out_acc.ap(), in_=acc[:])

    nc.compile()
    return nc, N_TILES


def get_nc():
    if "nc" not in _CACHE:
        _CACHE["nc"] = build_nc()
    return _CACHE["nc"]


def run_device(probs, lbls, trace=False, **run_kwargs):
    """Run the SPMD kernel; returns (per-core result dicts, BassKernelResults)."""
    from concourse import bass_utils

    nc, _ = get_nc()
    probs = np.ascontiguousarray(probs, dtype=np.float32)
    lbls = np.ascontiguousarray(lbls, dtype=np.int32)
    assert probs.shape == (N_TOTAL,) and lbls.shape == (N_TOTAL,)
    M = M_PER_CORE
    ident = np.eye(P, CH, dtype=np.float32)
    in_maps = [
        {"probs": probs[c * M:(c + 1) * M], "lbls": lbls[c * M:(c + 1) * M],
         "ident": ident}
        for c in range(N_CORES)
    ]
    res = bass_utils.run_bass_kernel_spmd(
        nc, in_maps, core_ids=list(range(N_CORES)), trace=trace, **run_kwargs)
    return res.results, res


def combine(results):
    """Combine per-core partial sums into the final -f1 scalar."""
    T = N_TILES
    Npos = Y = SxC = TP = 0.0
    for r in results:
        a = np.asarray(r["out_acc"], dtype=np.float64)
        Npos += a[:HEAD_P, 0].sum() + a[:, 1:T].sum()
        Y += a[:HEAD_P, T].sum() + a[:, T + 1:2 * T].sum()
        SxC += a[:, 2 * T].sum()
        TP += a[:, 2 * T + 1].sum()

    FP = Y - TP
    FN = Npos - SxC + TP
    precision = (TP + EPS) / (TP + FP + EPS)
    recall = (TP + EPS) / (TP + FN + EPS)
    f1 = 2.0 * precision * recall / (precision + recall)
    return np.float32(-f1)


def kernel(probs, lbls):
    results, _ = run_device(probs, lbls)
    return np.asarray(combine(results), dtype=np.float32)


if __name__ == "__main__":
    rng = np.random.default_rng(0)
    p = rng.uniform(0, 1, N_TOTAL).astype(np.float32)
    l = rng.integers(0, 2, N_TOTAL).astype(np.int32)
    out = kernel(p, l)
    print("kernel output:", out)
